# revision 3
# baseline (speedup 1.0000x reference)
"""Trainium2 raw-bass kernel for Mixtral SwiGLU MLP with HQQ 4-bit weights.

Tensor-parallel over the intermediate dim (14336 -> 1792 per core, 8 cores).
Host dequantizes weights; device runs explicit-ldweights matmul streams
(the Tile framework's ldweights-per-matmul costs ~2.3x on HW).

Structure per 2048-token block:
  UP:   w-stationary bf16: per (w-tile): 1 LDWEIGHTS + 4 matmuls (N=512
        x-moving), g -> psum 0-3, u -> psum 4-7; silu on ScalarE,
        h = silu(g)*u on VectorE -> fp8 (u pre-scaled 2^-15 via w3).
  DOWN: h-stationary fp8 x w2-moving fp8 (plain fp8 = bf16 rate): per
        h-tile: 1 LDWEIGHTS + 4 matmuls; w2 streamed in hid-halves;
        8 psum banks alternate by tt parity; drains -> bf16 out.
Host sums the 8 partial outputs (f32) and scales by 2^15.
"""

import os
import sys

for _p in ("/opt/trn_rl_repo", "/root/.axon_site/_ro/trn_rl_repo"):
    if os.path.isdir(_p) and _p not in sys.path:
        sys.path.insert(0, _p)

import ml_dtypes
import numpy as np

import concourse.bacc as bacc
import concourse.mybir as mybir
import concourse.bass as bass
from concourse.bass_utils import run_bass_kernel_spmd

BF16 = ml_dtypes.bfloat16
FP8 = ml_dtypes.float8_e4m3

N_CORES = 8
TOK = 4096
HID = 4096
INT = 14336
GS = 64

INT_SH = INT // N_CORES          # 1792
TB = 2048                        # token block
NB = TOK // TB                   # 2
I_TILES = INT_SH // 128          # 14
H_TILES = HID // 128             # 32
TBS = TB // 512                  # 4 moving blocks per weight tile
HQS = 4                          # hid 512-blocks per half
USCALE = 2.0 ** -15              # folded into w3 (bf16), undone on host

_CACHE = {}


def _build_nc(repeats=1, sigmoid_for_sim=False):
    key = ("nc", repeats, sigmoid_for_sim)
    if key in _CACHE:
        return _CACHE[key]

    nc = bacc.Bacc("TRN2", target_bir_lowering=False, debug=False)
    bf = mybir.dt.bfloat16
    f8 = mybir.dt.float8e4
    f32 = mybir.dt.float32
    Silu = (mybir.ActivationFunctionType.Sigmoid if sigmoid_for_sim
            else mybir.ActivationFunctionType.Silu)

    x_d = nc.dram_tensor("xt", [NB, 128, H_TILES, TB], bf, kind="ExternalInput")
    w1_d = nc.dram_tensor("w1t", [I_TILES, 128, H_TILES, 128], bf,
                          kind="ExternalInput")
    w3_d = nc.dram_tensor("w3t", [I_TILES, 128, H_TILES, 128], bf,
                          kind="ExternalInput")
    w2_d = nc.dram_tensor("w2t", [2, 128, I_TILES, TB], f8, kind="ExternalInput")
    out_d = nc.dram_tensor("out", [TOK, HID], bf, kind="ExternalOutput")

    xt_sb = nc.alloc_sbuf_tensor("xt_sb", [128, H_TILES, TB], bf)
    w1_sb = nc.alloc_sbuf_tensor("w1_sb", [128, H_TILES, 128], bf)
    w3_sb = nc.alloc_sbuf_tensor("w3_sb", [128, H_TILES, 128], bf)
    h_sb = nc.alloc_sbuf_tensor("h_sb", [128, I_TILES, TB], f8)
    w2_sb = nc.alloc_sbuf_tensor("w2_sb", [128, I_TILES, TB], f8)
    sil_sb = nc.alloc_sbuf_tensor("sil_sb", [128, 2, 512], bf)
    o_sb = nc.alloc_sbuf_tensor("o_sb", [128, 2, 1024], bf)
    ps = [nc.alloc_psum_tensor(f"ps{i}", [128, 512], f32) for i in range(8)]

    s_x = nc.alloc_semaphore("s_x")            # x-chunk DMAs (16 each)
    s_w13 = nc.alloc_semaphore("s_w13")        # w1/w3 tile DMAs (16 each)
    s_w1f = nc.alloc_semaphore("s_w1f")        # PE: g-loop(it) done (1/it)
    s_w3f = nc.alloc_semaphore("s_w3f")        # PE: u-loop(it) done (1/it)
    s_w2 = nc.alloc_semaphore("s_w2")          # w2 half DMAs (16 each)
    s_sil = nc.alloc_semaphore("s_sil")        # scalar: silu block done
    s_mul = nc.alloc_semaphore("s_mul")        # vector: h block done
    s_pedn = nc.alloc_semaphore("s_pedn")      # PE: down group (tt) done
    s_ordy = nc.alloc_semaphore("s_ordy")      # vector: o_sb chunk ready (2/grp)
    s_ofree = [nc.alloc_semaphore("s_ofree0"),
               nc.alloc_semaphore("s_ofree1")]  # out DMA done, by o_sb slot

    NBLK = NB * repeats            # total token blocks processed
    X_DMAS = 4                     # x chunks per block
    GRPS_PER_BLK = 2 * 16          # (hh, tt) groups per block
    MULS_PER_BLK = I_TILES * TBS   # vector h-muls per block

    def blocks():
        for rep in range(repeats):
            for b in range(NB):
                yield rep * NB + b, b

    with nc.Block() as block:

        @block.sync
        def _(sync: bass.BassEngine):
            for bi, b in blocks():
                if bi >= 1:
                    sync.wait_ge(s_w3f, I_TILES * bi)
                for q in range(X_DMAS):
                    hs = H_TILES // X_DMAS
                    sync.dma_start(
                        xt_sb[:, q * hs:(q + 1) * hs, :],
                        x_d[b, :, q * hs:(q + 1) * hs, :],
                    ).then_inc(s_x, 16)
                for it in range(I_TILES):
                    itg = bi * I_TILES + it
                    if itg >= 1:
                        sync.wait_ge(s_w1f, itg)
                    sync.dma_start(w1_sb[:], w1_d[it]).then_inc(s_w13, 16)
                    if itg >= 1:
                        sync.wait_ge(s_w3f, itg)
                    sync.dma_start(w3_sb[:], w3_d[it]).then_inc(s_w13, 16)
                for hh in range(2):
                    prev_groups = bi * GRPS_PER_BLK + hh * 16
                    if prev_groups > 0:
                        sync.wait_ge(s_pedn, prev_groups)
                    sync.dma_start(w2_sb[:], w2_d[hh]).then_inc(s_w2, 16)

        @block.tensor
        def _(tensor: bass.BassEngine):
            grp = 0
            for bi, b in blocks():
                # ---------------- UP ----------------
                tensor.wait_ge(s_x, 16 * X_DMAS * (bi + 1))
                if bi >= 1:
                    # all psum banks must be drained from previous down phase
                    tensor.wait_ge(s_ordy, 2 * bi * GRPS_PER_BLK)
                for it in range(I_TILES):
                    itg = bi * I_TILES + it
                    tensor.wait_ge(s_w13, 32 * (itg + 1))
                    if itg >= 1:
                        # scalar finished reading prev it's g/u banks
                        tensor.wait_ge(s_sil, TBS * itg)
                    last = None
                    for ht in range(H_TILES):
                        w = w1_sb[:, ht, :]
                        tensor.ldweights(w)
                        for tb in range(TBS):
                            last = tensor.matmul(
                                ps[tb][:], w, xt_sb[:, ht, tb * 512:(tb + 1) * 512],
                                start=(ht == 0), stop=(ht == H_TILES - 1))
                            last.ins.ldweights = False
                    last.then_inc(s_w1f, 1)
                    if itg >= 1:
                        # vector finished reading prev it's u banks
                        tensor.wait_ge(s_mul, TBS * itg)
                    for ht in range(H_TILES):
                        w = w3_sb[:, ht, :]
                        tensor.ldweights(w)
                        for tb in range(TBS):
                            last = tensor.matmul(
                                ps[4 + tb][:], w,
                                xt_sb[:, ht, tb * 512:(tb + 1) * 512],
                                start=(ht == 0), stop=(ht == H_TILES - 1))
                            last.ins.ldweights = False
                    last.then_inc(s_w3f, 1)
                # ---------------- DOWN ----------------
                tensor.wait_ge(s_mul, MULS_PER_BLK * (bi + 1))
                for hh in range(2):
                    tensor.wait_ge(s_w2, 16 * (2 * bi + hh + 1))
                    for tt in range(16):
                        if grp >= 2:
                            tensor.wait_ge(s_ordy, 2 * (grp - 1))
                        bs = (tt % 2) * 4
                        last = None
                        for it in range(I_TILES):
                            h_t = h_sb[:, it, tt * 128:(tt + 1) * 128]
                            tensor.ldweights(h_t)
                            for hq in range(HQS):
                                last = tensor.matmul(
                                    ps[bs + hq][:], h_t,
                                    w2_sb[:, it, hq * 512:(hq + 1) * 512],
                                    start=(it == 0), stop=(it == I_TILES - 1))
                                last.ins.ldweights = False
                        last.then_inc(s_pedn, 1)
                        grp += 1

        @block.scalar
        def _(scalar: bass.BassEngine):
            for bi, b in blocks():
                for it in range(I_TILES):
                    itg = bi * I_TILES + it
                    scalar.wait_ge(s_w1f, itg + 1)
                    for tb in range(TBS):
                        j = itg * TBS + tb
                        if j >= 2:
                            scalar.wait_ge(s_mul, j - 1)
                        scalar.activation(
                            sil_sb[:, j % 2, :], ps[tb][:], Silu
                        ).then_inc(s_sil, 1)

        @block.vector
        def _(vector: bass.BassEngine):
            dma_i = 0
            for bi, b in blocks():
                for it in range(I_TILES):
                    itg = bi * I_TILES + it
                    vector.wait_ge(s_w3f, itg + 1)
                    for tb in range(TBS):
                        j = itg * TBS + tb
                        vector.wait_ge(s_sil, j + 1)
                        vector.tensor_mul(
                            h_sb[:, it, tb * 512:(tb + 1) * 512],
                            sil_sb[:, j % 2, :], ps[4 + tb][:],
                        ).then_inc(s_mul, 1)
                for hh in range(2):
                    for tt in range(16):
                        g = bi * GRPS_PER_BLK + hh * 16 + tt
                        vector.wait_ge(s_pedn, g + 1)
                        bs = (tt % 2) * 4
                        last = None
                        for c in range(2):
                            if dma_i >= 2:
                                vector.wait_ge(s_ofree[dma_i % 2],
                                               16 * (dma_i // 2))
                            for k in range(2):
                                last = vector.tensor_copy(
                                    o_sb[:, dma_i % 2, k * 512:(k + 1) * 512],
                                    ps[bs + c * 2 + k][:])
                            last.then_inc(s_ordy, 1)
                            dma_i += 1

        @block.gpsimd
        def _(gpsimd: bass.BassGpSimd):
            dma_i = 0
            for bi, b in blocks():
                for hh in range(2):
                    for tt in range(16):
                        for c in range(2):
                            gpsimd.wait_ge(s_ordy, dma_i + 1)
                            rows = slice(b * TB + tt * 128, b * TB + (tt + 1) * 128)
                            col0 = hh * 2048 + c * 1024
                            gpsimd.dma_start(
                                out_d[rows, col0:col0 + 1024],
                                o_sb[:, dma_i % 2, :],
                            ).then_inc(s_ofree[dma_i % 2], 16)
                            dma_i += 1
            gpsimd.wait_ge(s_ofree[0], 16 * (dma_i // 2))
            gpsimd.wait_ge(s_ofree[1], 16 * (dma_i // 2))

    nc.compile()
    _CACHE[key] = nc
    return nc


def _dequant(q, s, z):
    out, inp = q.shape
    g = inp // GS
    qf = np.asarray(q, np.float32).reshape(out, g, GS)
    w = (qf - np.asarray(z, np.float32)[:, :, None]) * \
        np.asarray(s, np.float32)[:, :, None]
    return w.reshape(out, inp)


def _prep_in_maps(hidden_states, w1_q, w1_scale, w1_zero, w3_q, w3_scale,
                  w3_zero, w2_q, w2_scale, w2_zero):
    x = np.asarray(hidden_states, np.float32)
    # xt[b, p, a, t] = x[b*TB + t, a*128 + p]
    xt = np.ascontiguousarray(
        x.astype(BF16).reshape(NB, TB, H_TILES, 128).transpose(0, 3, 2, 1))

    def up_shard(q, s, z, c, scale):
        rows = slice(c * INT_SH, (c + 1) * INT_SH)
        wd = _dequant(q[rows], s[rows], z[rows]) * scale
        wd = wd.astype(BF16)
        # w1t[it, p, a, i] = wd[it*128 + i, a*128 + p]
        return np.ascontiguousarray(
            wd.reshape(I_TILES, 128, H_TILES, 128).transpose(0, 3, 2, 1))

    def down_shard(q, s, z, c):
        cols = slice(c * INT_SH, (c + 1) * INT_SH)
        gsl = slice(c * (INT_SH // GS), (c + 1) * (INT_SH // GS))
        wd = _dequant(np.ascontiguousarray(q[:, cols]), s[:, gsl],
                      z[:, gsl])                       # [HID, INT_SH]
        wd8 = np.clip(wd, -240, 240).astype(FP8)
        # w2t[hh, p, it, hcol] = wd[hh*2048 + hcol, it*128 + p]
        return np.ascontiguousarray(
            wd8.reshape(2, TB, I_TILES, 128).transpose(0, 3, 2, 1))

    in_maps = []
    for c in range(N_CORES):
        in_maps.append({
            "xt": xt,
            "w1t": up_shard(w1_q, w1_scale, w1_zero, c, 1.0),
            "w3t": up_shard(w3_q, w3_scale, w3_zero, c, USCALE),
            "w2t": down_shard(w2_q, w2_scale, w2_zero, c),
        })
    return in_maps


def kernel(**inputs):
    nc = _build_nc()
    in_maps = _prep_in_maps(**inputs)
    res = run_bass_kernel_spmd(nc, in_maps, core_ids=list(range(N_CORES)))
    out = np.zeros((TOK, HID), np.float32)
    for c in range(N_CORES):
        out += res.results[c]["out"].astype(np.float32)
    return (out * np.float32(1.0 / USCALE)).astype(np.float32, copy=False)


if __name__ == "__main__":
    rng = np.random.default_rng(0)
    ins = {
        "hidden_states": rng.standard_normal((TOK, HID)).astype(np.float32),
        "w1_q": rng.integers(0, 16, (INT, HID)).astype(np.int32),
        "w1_scale": rng.random((INT, HID // GS)).astype(np.float32),
        "w1_zero": rng.random((INT, HID // GS)).astype(np.float32),
        "w3_q": rng.integers(0, 16, (INT, HID)).astype(np.int32),
        "w3_scale": rng.random((INT, HID // GS)).astype(np.float32),
        "w3_zero": rng.random((INT, HID // GS)).astype(np.float32),
        "w2_q": rng.integers(0, 16, (HID, INT)).astype(np.int32),
        "w2_scale": rng.random((HID, INT // GS)).astype(np.float32),
        "w2_zero": rng.random((HID, INT // GS)).astype(np.float32),
    }
    out = kernel(**ins)
    print("out", out.shape, out.dtype, float(np.abs(out).max()))

    # quick numpy check
    def deq(q, s, z):
        return _dequant(q, s, z)
    w1 = deq(ins["w1_q"], ins["w1_scale"], ins["w1_zero"])
    w3 = deq(ins["w3_q"], ins["w3_scale"], ins["w3_zero"])
    w2 = deq(ins["w2_q"], ins["w2_scale"], ins["w2_zero"])
    xx = ins["hidden_states"]
    g = xx @ w1.T
    u = xx @ w3.T
    h = np.where(g > 30, g, g / (1 + np.exp(-np.clip(g, -80, 30)))) * u
    ref = h @ w2.T
    err = np.abs(out - ref)
    print("relmax", err.max() / np.abs(ref).max())


# revision 4
# speedup vs baseline: 3.9518x; 3.9518x over previous
"""Trainium2 raw-bass kernel for Mixtral SwiGLU MLP with HQQ 4-bit weights.

Tensor-parallel over the intermediate dim (14336 -> 1792 per core, 8 cores).
Host dequantizes weights; device runs explicit-ldweights matmul streams
(the Tile framework's ldweights-per-matmul costs ~2.3x on HW).

Structure per 2048-token block:
  UP:   w-stationary bf16: per (w-tile): 1 LDWEIGHTS + 4 matmuls (N=512
        x-moving), g -> psum 0-3, u -> psum 4-7; silu on ScalarE,
        h = silu(g)*u on VectorE -> fp8 (u pre-scaled 2^-15 via w3).
  DOWN: h-stationary fp8 x w2-moving fp8 (plain fp8 = bf16 rate): per
        h-tile: 1 LDWEIGHTS + 4 matmuls; w2 streamed in hid-halves;
        8 psum banks alternate by tt parity; drains -> bf16 out.
Host sums the 8 partial outputs (f32) and scales by 2^15.
"""

import os
import sys

for _p in ("/opt/trn_rl_repo", "/root/.axon_site/_ro/trn_rl_repo"):
    if os.path.isdir(_p) and _p not in sys.path:
        sys.path.insert(0, _p)

import ml_dtypes
import numpy as np

import concourse.bacc as bacc
import concourse.mybir as mybir
import concourse.bass as bass
from concourse.bass_utils import run_bass_kernel_spmd

BF16 = ml_dtypes.bfloat16
FP8 = ml_dtypes.float8_e4m3

N_CORES = 8
TOK = 4096
HID = 4096
INT = 14336
GS = 64

INT_SH = INT // N_CORES          # 1792
TB = 2048                        # token block
NB = TOK // TB                   # 2
I_TILES = INT_SH // 128          # 14
H_TILES = HID // 128             # 32
TBS = TB // 512                  # 4 moving blocks per weight tile
HQS = 4                          # hid 512-blocks per half
USCALE = 2.0 ** -15              # folded into w3 (bf16), undone on host

_CACHE = {}


def _build_nc(repeats=1, sigmoid_for_sim=False):
    key = ("nc", repeats, sigmoid_for_sim)
    if key in _CACHE:
        return _CACHE[key]

    nc = bacc.Bacc("TRN2", target_bir_lowering=False, debug=False)
    bf = mybir.dt.bfloat16
    f8 = mybir.dt.float8e4
    f32 = mybir.dt.float32
    Silu = (mybir.ActivationFunctionType.Sigmoid if sigmoid_for_sim
            else mybir.ActivationFunctionType.Silu)

    x_d = nc.dram_tensor("xt", [NB, 128, H_TILES, TB], bf, kind="ExternalInput")
    w1_d = nc.dram_tensor("w1t", [I_TILES, 128, H_TILES, 128], bf,
                          kind="ExternalInput")
    w3_d = nc.dram_tensor("w3t", [I_TILES, 128, H_TILES, 128], bf,
                          kind="ExternalInput")
    w2_d = nc.dram_tensor("w2t", [2, 128, I_TILES // 2, 2, TB], f8,
                          kind="ExternalInput")
    out_d = nc.dram_tensor("out", [TOK, HID], bf, kind="ExternalOutput")

    xt_sb = nc.alloc_sbuf_tensor("xt_sb", [128, H_TILES, TB], bf)
    w1_sb = nc.alloc_sbuf_tensor("w1_sb", [128, H_TILES, 128], bf)
    w3_sb = nc.alloc_sbuf_tensor("w3_sb", [128, H_TILES, 128], bf)
    h_sb = nc.alloc_sbuf_tensor("h_sb", [128, I_TILES, TB], f8)
    w2_sb = nc.alloc_sbuf_tensor("w2_sb", [128, I_TILES // 2, 2, TB], f8)
    sil_sb = nc.alloc_sbuf_tensor("sil_sb", [128, 2, 512], bf)
    o_sb = nc.alloc_sbuf_tensor("o_sb", [128, 2, 1024], bf)
    ps = [nc.alloc_psum_tensor(f"ps{i}", [128, 512], f32) for i in range(8)]

    s_x = nc.alloc_semaphore("s_x")            # x-chunk DMAs (16 each)
    s_w13 = nc.alloc_semaphore("s_w13")        # w1/w3 tile DMAs (16 each)
    s_w1f = nc.alloc_semaphore("s_w1f")        # PE: g-loop(it) done (1/it)
    s_w3f = nc.alloc_semaphore("s_w3f")        # PE: u-loop(it) done (1/it)
    s_w2 = nc.alloc_semaphore("s_w2")          # w2 half DMAs (16 each)
    s_sil = nc.alloc_semaphore("s_sil")        # scalar: silu block done
    s_mul = nc.alloc_semaphore("s_mul")        # vector: h block done
    s_pedn = nc.alloc_semaphore("s_pedn")      # PE: down group (tt) done
    s_ordy = nc.alloc_semaphore("s_ordy")      # vector: o_sb chunk ready (2/grp)
    s_ofree = [nc.alloc_semaphore("s_ofree0"),
               nc.alloc_semaphore("s_ofree1")]  # out DMA done, by o_sb slot

    NBLK = NB * repeats            # total token blocks processed
    X_DMAS = 4                     # x chunks per block
    GRPS_PER_BLK = 2 * 16          # (hh, tt) groups per block
    MULS_PER_BLK = I_TILES * TBS   # vector h-muls per block

    def blocks():
        for rep in range(repeats):
            for b in range(NB):
                yield rep * NB + b, b

    with nc.Block() as block:

        @block.sync
        def _(sync: bass.BassEngine):
            for bi, b in blocks():
                if bi >= 1:
                    sync.wait_ge(s_w3f, I_TILES * bi)
                for q in range(X_DMAS):
                    hs = H_TILES // X_DMAS
                    sync.dma_start(
                        xt_sb[:, q * hs:(q + 1) * hs, :],
                        x_d[b, :, q * hs:(q + 1) * hs, :],
                    ).then_inc(s_x, 16)
                for it in range(I_TILES):
                    itg = bi * I_TILES + it
                    if itg >= 1:
                        sync.wait_ge(s_w1f, itg)
                    sync.dma_start(w1_sb[:], w1_d[it]).then_inc(s_w13, 16)
                    if itg >= 1:
                        sync.wait_ge(s_w3f, itg)
                    sync.dma_start(w3_sb[:], w3_d[it]).then_inc(s_w13, 16)
                for hh in range(2):
                    prev_groups = bi * GRPS_PER_BLK + hh * 16
                    if prev_groups > 0:
                        sync.wait_ge(s_pedn, prev_groups)
                    sync.dma_start(w2_sb[:], w2_d[hh]).then_inc(s_w2, 16)

        @block.tensor
        def _(tensor: bass.BassEngine):
            grp = 0
            for bi, b in blocks():
                # ---------------- UP ----------------
                tensor.wait_ge(s_x, 16 * X_DMAS * (bi + 1))
                if bi >= 1:
                    # all psum banks must be drained from previous down phase
                    tensor.wait_ge(s_ordy, 2 * bi * GRPS_PER_BLK)
                for it in range(I_TILES):
                    itg = bi * I_TILES + it
                    tensor.wait_ge(s_w13, 32 * (itg + 1))
                    if itg >= 1:
                        # scalar finished reading prev it's g/u banks
                        tensor.wait_ge(s_sil, TBS * itg)
                    last = None
                    for ht in range(H_TILES):
                        w = w1_sb[:, ht, :]
                        tensor.ldweights(w)
                        for tb in range(TBS):
                            last = tensor.matmul(
                                ps[tb][:], w, xt_sb[:, ht, tb * 512:(tb + 1) * 512],
                                start=(ht == 0), stop=(ht == H_TILES - 1))
                            last.ins.ldweights = False
                    last.then_inc(s_w1f, 1)
                    if itg >= 1:
                        # vector finished reading prev it's u banks
                        tensor.wait_ge(s_mul, TBS * itg)
                    for ht in range(H_TILES):
                        w = w3_sb[:, ht, :]
                        tensor.ldweights(w)
                        for tb in range(TBS):
                            last = tensor.matmul(
                                ps[4 + tb][:], w,
                                xt_sb[:, ht, tb * 512:(tb + 1) * 512],
                                start=(ht == 0), stop=(ht == H_TILES - 1))
                            last.ins.ldweights = False
                    last.then_inc(s_w3f, 1)
                # ---------------- DOWN ----------------
                tensor.wait_ge(s_mul, MULS_PER_BLK * (bi + 1))
                for hh in range(2):
                    tensor.wait_ge(s_w2, 16 * (2 * bi + hh + 1))
                    for tt in range(16):
                        if grp >= 2:
                            tensor.wait_ge(s_ordy, 2 * (grp - 1))
                        bs = (tt % 2) * 4
                        last = None
                        DR = mybir.MatmulPerfMode.DoubleRow
                        for itp in range(I_TILES // 2):
                            h_t = h_sb[:, 2 * itp:2 * itp + 2,
                                       tt * 128:(tt + 1) * 128]
                            tensor.ldweights(h_t, perf_mode=DR)
                            for hq in range(HQS):
                                last = tensor.matmul(
                                    ps[bs + hq][:], h_t,
                                    w2_sb[:, itp, :, hq * 512:(hq + 1) * 512],
                                    start=(itp == 0),
                                    stop=(itp == I_TILES // 2 - 1),
                                    perf_mode=DR)
                                last.ins.ldweights = False
                        last.then_inc(s_pedn, 1)
                        grp += 1

        @block.scalar
        def _(scalar: bass.BassEngine):
            for bi, b in blocks():
                for it in range(I_TILES):
                    itg = bi * I_TILES + it
                    scalar.wait_ge(s_w1f, itg + 1)
                    for tb in range(TBS):
                        j = itg * TBS + tb
                        if j >= 2:
                            scalar.wait_ge(s_mul, j - 1)
                        scalar.activation(
                            sil_sb[:, j % 2, :], ps[tb][:], Silu
                        ).then_inc(s_sil, 1)

        @block.vector
        def _(vector: bass.BassEngine):
            dma_i = 0
            for bi, b in blocks():
                for it in range(I_TILES):
                    itg = bi * I_TILES + it
                    vector.wait_ge(s_w3f, itg + 1)
                    for tb in range(TBS):
                        j = itg * TBS + tb
                        vector.wait_ge(s_sil, j + 1)
                        vector.tensor_mul(
                            h_sb[:, it, tb * 512:(tb + 1) * 512],
                            sil_sb[:, j % 2, :], ps[4 + tb][:],
                        ).then_inc(s_mul, 1)
                for hh in range(2):
                    for tt in range(16):
                        g = bi * GRPS_PER_BLK + hh * 16 + tt
                        vector.wait_ge(s_pedn, g + 1)
                        bs = (tt % 2) * 4
                        last = None
                        for c in range(2):
                            if dma_i >= 2:
                                vector.wait_ge(s_ofree[dma_i % 2],
                                               16 * (dma_i // 2))
                            for k in range(2):
                                last = vector.tensor_copy(
                                    o_sb[:, dma_i % 2, k * 512:(k + 1) * 512],
                                    ps[bs + c * 2 + k][:])
                            last.then_inc(s_ordy, 1)
                            dma_i += 1

        @block.gpsimd
        def _(gpsimd: bass.BassGpSimd):
            dma_i = 0
            for bi, b in blocks():
                for hh in range(2):
                    for tt in range(16):
                        for c in range(2):
                            gpsimd.wait_ge(s_ordy, dma_i + 1)
                            rows = slice(b * TB + tt * 128, b * TB + (tt + 1) * 128)
                            col0 = hh * 2048 + c * 1024
                            gpsimd.dma_start(
                                out_d[rows, col0:col0 + 1024],
                                o_sb[:, dma_i % 2, :],
                            ).then_inc(s_ofree[dma_i % 2], 16)
                            dma_i += 1
            gpsimd.wait_ge(s_ofree[0], 16 * (dma_i // 2))
            gpsimd.wait_ge(s_ofree[1], 16 * (dma_i // 2))

    nc.compile()
    _CACHE[key] = nc
    return nc


def _dequant(q, s, z):
    out, inp = q.shape
    g = inp // GS
    qf = np.asarray(q, np.float32).reshape(out, g, GS)
    w = (qf - np.asarray(z, np.float32)[:, :, None]) * \
        np.asarray(s, np.float32)[:, :, None]
    return w.reshape(out, inp)


def _prep_in_maps(hidden_states, w1_q, w1_scale, w1_zero, w3_q, w3_scale,
                  w3_zero, w2_q, w2_scale, w2_zero):
    x = np.asarray(hidden_states, np.float32)
    # xt[b, p, a, t] = x[b*TB + t, a*128 + p]
    xt = np.ascontiguousarray(
        x.astype(BF16).reshape(NB, TB, H_TILES, 128).transpose(0, 3, 2, 1))

    def up_shard(q, s, z, c, scale):
        rows = slice(c * INT_SH, (c + 1) * INT_SH)
        wd = _dequant(q[rows], s[rows], z[rows]) * scale
        wd = wd.astype(BF16)
        # w1t[it, p, a, i] = wd[it*128 + i, a*128 + p]
        return np.ascontiguousarray(
            wd.reshape(I_TILES, 128, H_TILES, 128).transpose(0, 3, 2, 1))

    def down_shard(q, s, z, c):
        cols = slice(c * INT_SH, (c + 1) * INT_SH)
        gsl = slice(c * (INT_SH // GS), (c + 1) * (INT_SH // GS))
        wd = _dequant(np.ascontiguousarray(q[:, cols]), s[:, gsl],
                      z[:, gsl])                       # [HID, INT_SH]
        wd8 = np.clip(wd, -240, 240).astype(FP8)
        # w2t[hh, p, itp, j, hcol] = wd[hh*2048 + hcol, (2*itp+j)*128 + p]
        return np.ascontiguousarray(
            wd8.reshape(2, TB, I_TILES // 2, 2, 128).transpose(0, 4, 2, 3, 1))

    in_maps = []
    for c in range(N_CORES):
        in_maps.append({
            "xt": xt,
            "w1t": up_shard(w1_q, w1_scale, w1_zero, c, 1.0),
            "w3t": up_shard(w3_q, w3_scale, w3_zero, c, USCALE),
            "w2t": down_shard(w2_q, w2_scale, w2_zero, c),
        })
    return in_maps


def kernel(**inputs):
    nc = _build_nc()
    in_maps = _prep_in_maps(**inputs)
    res = run_bass_kernel_spmd(nc, in_maps, core_ids=list(range(N_CORES)))
    out = np.zeros((TOK, HID), np.float32)
    for c in range(N_CORES):
        out += res.results[c]["out"].astype(np.float32)
    return (out * np.float32(1.0 / USCALE)).astype(np.float32, copy=False)


if __name__ == "__main__":
    rng = np.random.default_rng(0)
    ins = {
        "hidden_states": rng.standard_normal((TOK, HID)).astype(np.float32),
        "w1_q": rng.integers(0, 16, (INT, HID)).astype(np.int32),
        "w1_scale": rng.random((INT, HID // GS)).astype(np.float32),
        "w1_zero": rng.random((INT, HID // GS)).astype(np.float32),
        "w3_q": rng.integers(0, 16, (INT, HID)).astype(np.int32),
        "w3_scale": rng.random((INT, HID // GS)).astype(np.float32),
        "w3_zero": rng.random((INT, HID // GS)).astype(np.float32),
        "w2_q": rng.integers(0, 16, (HID, INT)).astype(np.int32),
        "w2_scale": rng.random((HID, INT // GS)).astype(np.float32),
        "w2_zero": rng.random((HID, INT // GS)).astype(np.float32),
    }
    out = kernel(**ins)
    print("out", out.shape, out.dtype, float(np.abs(out).max()))

    # quick numpy check
    def deq(q, s, z):
        return _dequant(q, s, z)
    w1 = deq(ins["w1_q"], ins["w1_scale"], ins["w1_zero"])
    w3 = deq(ins["w3_q"], ins["w3_scale"], ins["w3_zero"])
    w2 = deq(ins["w2_q"], ins["w2_scale"], ins["w2_zero"])
    xx = ins["hidden_states"]
    g = xx @ w1.T
    u = xx @ w3.T
    h = np.where(g > 30, g, g / (1 + np.exp(-np.clip(g, -80, 30)))) * u
    ref = h @ w2.T
    err = np.abs(out - ref)
    print("relmax", err.max() / np.abs(ref).max())
